# revision 4
# baseline (speedup 1.0000x reference)
"""ActiveShiftLayer Trainium2 kernel (v2).

out[n,c,h,w] = bilinear sample of x[n,c, h+alpha_c, w+beta_c], zero outside
the spatial extent.

alpha,beta in [-1,1) => floor in {-1,0}, so the bilinear sample is a
separable 3-tap convolution along H then W with per-channel tap weights;
per channel only 2 of the 3 taps are nonzero in each direction.

Key optimizations over v1 (83.5us):
- fp16 input: x is cast to fp16 on the host, halving HBM read traffic
  (error budget 2e-2 >> fp16's ~1e-4).
- channel sort: channels are permuted on the host so the 128-channel
  partition blocks are grouped by floor(alpha) (V-tap set) and, within
  that, by floor(beta) (H-tap side). A block whose channels all share
  floor(alpha) needs only 2 V-tap matmul passes instead of 3; the H outer
  taps are issued per contiguous partition run with ONLY the nonzero side,
  halving DVE/Pool elementwise work.
- no PSUM->SBUF copy pass: the H stage reads vt straight from PSUM.
  ScalarE applies the center tap (PSUM -> fp16 OUT with per-channel
  scale); the outer taps are scalar_tensor_tensor accumulates whose
  pieces alternate between VectorE and GpSimd (Pool).
- fp16 OUT in SBUF: stores are plain HWDGE triggered from the Scalar
  queue; the sync queue only carries loads.

The Bass program is built after seeing shift_param (tap sets / partition
runs are data-dependent) and cached by that structure.

Sharding: data-parallel over batch (N=32 -> 4 per core), each core also
splits C=256 into two partition blocks -> 8 tiles of [128 channels
(partitions), 56x56 plane (free dim)] per core. Pure SPMD, no collectives.
"""

import os
import numpy as np

N, C, H, W = 32, 256, 56, 56
NCORES = 8
NSH = N // NCORES  # batches per core
P = 128
CB = C // P        # channel blocks
HW = H * W         # 3136
CHUNK = 512        # one PSUM bank of f32 per matmul
XLEN = W + HW + W + 16  # guard row above/below + pad
# fraction of pieces whose outer taps run on Pool (GpSimd) instead of DVE:
# piece k goes to Pool iff (k % POOL_MOD) < POOL_TAKE.
# NOTE: GPSIMD cannot access PSUM (walrus birverifier) — Pool STT would need
# an SBUF vt copy, so default is all-DVE.
POOL_MOD = int(os.environ.get("ASL_POOL_MOD", "2"))
POOL_TAKE = int(os.environ.get("ASL_POOL_TAKE", "0"))

_CACHE = {}


def _build_nc(taps, runs):
    """taps: per-cb tuple of dy offsets needed (subset of (-1,0,1)).
    runs: per-cb tuple of (p0, p1, side) partition runs; side 0 = left tap
    (dx=-1, weight col 1), side 1 = right tap (dx=+1, weight col 2)."""
    import concourse.bacc as bacc
    import concourse.mybir as mybir
    import concourse.tile as tile

    f32 = mybir.dt.float32
    f16 = mybir.dt.float16
    mult = mybir.AluOpType.mult
    add = mybir.AluOpType.add
    act_copy = mybir.ActivationFunctionType.Copy

    nc = bacc.Bacc()
    xs = nc.dram_tensor("xs", [NSH, C, H, W], f16, kind="ExternalInput")
    # wd[cb] = per-tap diag matrices for channels cb*128..cb*128+127
    NT0 = max(len(t) for t in taps)
    wd = nc.dram_tensor("wd", [CB, P, NT0 * P], f16, kind="ExternalInput")
    # wv[cb] columns: [wh_0, wh_m1, wh_p1]
    wv = nc.dram_tensor("wv", [CB, P, 3], f32, kind="ExternalInput")
    ys = nc.dram_tensor("ys", [NSH, C, H, W], f16, kind="ExternalOutput")

    with tile.TileContext(nc) as tc:
        with tc.tile_pool(name="wp", bufs=1) as wp, \
             tc.tile_pool(name="op", bufs=4) as opool, \
             tc.tile_pool(name="ps", bufs=2, space="PSUM") as ppool:

            wdt = []
            wvt = []

            def load_weights(cb):
                t = wp.tile([P, NT0 * P], f16, tag=f"wd{cb}")
                nc.sync.dma_start(t[:], wd[cb])
                wdt.append(t)
                v = wp.tile([P, 3], f32, tag=f"wv{cb}")
                nc.sync.dma_start(v[:], wv[cb])
                wvt.append(v)

            # persistent X buffers: guards are zeroed once, loads only
            # rewrite the middle region
            NXBUF = 4
            xbufs = []
            for i in range(NXBUF):
                xb = wp.tile([P, XLEN], f16, tag=f"X{i}")
                nc.vector.memset(xb[:, 0:W], 0.0)
                nc.vector.memset(xb[:, W + HW:W + HW + W], 0.0)
                xbufs.append(xb)

            load_weights(0)

            tiles = [(n, cb) for n in range(NSH) for cb in range(CB)]
            NT = len(tiles)

            # row boundaries of the compute pieces per tile: quarters for
            # the first/last tile (fast pipeline fill/drain), halves
            # otherwise
            def bounds(idx):
                if idx == 0:
                    return [0, 4, 9, 16, 28, 42, 56]
                if idx == NT - 1:
                    return [0, 14, 28, 42, 49, 56]
                return [0, 28, 56]

            def issue_load(idx):
                # loads are segmented so piece i only depends on segments
                # 0..i
                ln, lcb = tiles[idx]
                lcs = slice(lcb * P, (lcb + 1) * P)
                X = xbufs[idx % NXBUF]
                xflat = xs[ln, lcs, :, :].rearrange("p h w -> p (h w)")
                b = bounds(idx)
                cuts = [min(r + 1, H) for r in b[1:-1]] + [H]
                r0 = 0
                for r1 in cuts:
                    if r1 > r0:
                        nc.sync.dma_start(X[:, W + r0 * W:W + r1 * W],
                                          xflat[:, r0 * W:r1 * W])
                    r0 = r1
                return X

            xtiles = {}
            xtiles[0] = issue_load(0)
            load_weights(1)
            xtiles[1] = issue_load(1)
            xtiles[2] = issue_load(2)

            pc = 0  # global piece counter for DVE/Pool alternation
            for tidx, (n, cb) in enumerate(tiles):
                wvc = wvt[cb]
                cs = slice(cb * P, (cb + 1) * P)
                if tidx + 3 < NT:
                    xtiles[tidx + 3] = issue_load(tidx + 3)
                X = xtiles.pop(tidx)

                OUT = opool.tile([P, HW], f16)

                tb = bounds(tidx)
                for rr0, rr1 in zip(tb[:-1], tb[1:]):
                    p0 = rr0 * W
                    PZ = (rr1 - rr0) * W
                    HR = rr1 - rr0
                    # V-stage on TensorE: accumulating diag matmuls, taps
                    # at row offsets dy*56 into guarded X
                    PS = ppool.tile([P, 4 * CHUNK], f32, tag="ps")
                    tcb = taps[cb]
                    for ti, dy in enumerate(tcb):
                        for c0 in range(0, PZ, CHUNK):
                            cn = min(CHUNK, PZ - c0)
                            o = W + p0 + c0 + dy * W
                            nc.tensor.matmul(
                                PS[:, c0:c0 + cn],
                                wdt[cb][:, ti * P:(ti + 1) * P],
                                X[:, o:o + cn],
                                start=(ti == 0), stop=(ti == len(tcb) - 1))

                    op = OUT[:, p0:p0 + PZ]
                    o2 = op.rearrange("p (h w) -> p h w", w=W)
                    ps2 = PS[:, 0:PZ].rearrange("p (h w) -> p h w", w=W)

                    # H-stage center tap on ScalarE: PSUM -> fp16 OUT with
                    # per-channel scale
                    nc.scalar.activation(op, PS[:, 0:PZ], act_copy,
                                         scale=wvc[:, 0:1])

                    # outer taps: one STT per contiguous partition run,
                    # only the nonzero side; whole piece alternates
                    # between Pool and DVE
                    eng = nc.gpsimd if (pc % POOL_MOD) < POOL_TAKE \
                        else nc.vector
                    pc += 1
                    for (a, b2, side) in runs[cb]:
                        if side == 0:
                            # left: out[w] += wh_m1 * vt[w-1], w in 1..55
                            eng.scalar_tensor_tensor(
                                o2[a:b2, :, 1:W], ps2[a:b2, :, 0:W - 1],
                                wvc[a:b2, 1:2], o2[a:b2, :, 1:W],
                                op0=mult, op1=add)
                        else:
                            # right: out[w] += wh_p1 * vt[w+1], w in 0..54
                            eng.scalar_tensor_tensor(
                                o2[a:b2, :, 0:W - 1], ps2[a:b2, :, 1:W],
                                wvc[a:b2, 2:3], o2[a:b2, :, 0:W - 1],
                                op0=mult, op1=add)

                    # store this piece (fp16 HWDGE on the Scalar queue)
                    nc.scalar.dma_start(ys[n, cs, rr0:rr1, :], o2)
    nc.finalize()
    return nc


def _tap_weights(shift):
    """Per-channel 3-tap weights over offsets {-1,0,1} for shift in [-1,1)."""
    f = np.floor(shift)
    t = (shift - f).astype(np.float32)
    assert np.all((f == -1) | (f == 0)), "shift outside [-1,1) unsupported"
    w_m1 = np.where(f == -1, 1 - t, 0).astype(np.float32)
    w_0 = np.where(f == -1, t, 1 - t).astype(np.float32)
    w_p1 = np.where(f == 0, t, 0).astype(np.float32)
    return w_m1, w_0, w_p1


def _plan(sp):
    """Channel permutation + per-block structure from shift_param."""
    fa = np.floor(sp[:, 0]).astype(np.int64)  # alpha: H shift group
    fb = np.floor(sp[:, 1]).astype(np.int64)  # beta: W shift group
    # sort by (alpha group, beta group); -1 group first
    perm = np.lexsort((fb, fa))
    fa_s, fb_s = fa[perm], fb[perm]

    taps = []
    runs = []
    for cb in range(CB):
        cs = slice(cb * P, (cb + 1) * P)
        g = fa_s[cs]
        t = []
        if np.any(g == -1):
            t += [-1, 0]
        if np.any(g == 0):
            if 0 not in t:
                t.append(0)
            t.append(1)
        taps.append(tuple(t))
        # 32-aligned partition ranges per beta side (PSUM partition access
        # must be quadrant-aligned). A range may include partitions of the
        # other side: their weight for this side is exactly 0, so the
        # accumulate is a no-op there.
        sides = (fb_s[cs] == 0).astype(np.int64)  # 0 = left (fb==-1)
        r = []
        for side in (0, 1):
            blocks = sorted({p // 32 for p in range(P)
                             if sides[p] == side})
            a = None
            prev = None
            for b in blocks + [None]:
                if b is not None and prev is not None and b == prev + 1:
                    prev = b
                    continue
                if a is not None:
                    r.append((a * 32, (prev + 1) * 32, side))
                a = b
                prev = b
        runs.append(tuple(sorted(r)))
    return perm, tuple(taps), tuple(runs)


def _host_weights(sp, perm, taps):
    sps = sp[perm]
    wh_m1, wh_0, wh_p1 = _tap_weights(sps[:, 1])  # beta: W shift
    wv_m1, wv_0, wv_p1 = _tap_weights(sps[:, 0])  # alpha: H shift
    vtap = {-1: wv_m1, 0: wv_0, 1: wv_p1}
    NT0 = max(len(t) for t in taps)
    wd = np.zeros((CB, NT0, P, P), np.float32)
    for cb in range(CB):
        cs = slice(cb * P, (cb + 1) * P)
        for ti, dy in enumerate(taps[cb]):
            wd[cb, ti] = np.diag(vtap[dy][cs])
    wd = wd.transpose(0, 2, 1, 3).reshape(CB, P, NT0 * P)
    wd = np.ascontiguousarray(wd.astype(np.float16))
    wvv = np.stack([wh_0, wh_m1, wh_p1], axis=1).astype(np.float32)
    wvv = np.ascontiguousarray(wvv.reshape(CB, P, 3))
    return wd, wvv


def _install_trace_shim():
    """Dev-only: register the NTFF profile hook this container's antenv lacks,
    and stub out the artifact upload (zero-egress container)."""
    import sys
    import types

    try:
        from antenv.axon_hooks import get_axon_ntff_profile_hook  # noqa: F401
    except ImportError:
        from trn_agent_boot.trn_boot import _ntff_profile_via_ctypes

        hook = _ntff_profile_via_ctypes("/opt/axon/libaxon_pjrt.so")
        mod = types.ModuleType("antenv.axon_hooks")
        mod.get_axon_ntff_profile_hook = lambda: hook
        mod.set_axon_ntff_profile_hook = lambda h: None
        import antenv

        sys.modules["antenv.axon_hooks"] = mod
        antenv.axon_hooks = mod

    import concourse.bass_utils as bu

    bu.upload_artifacts = lambda tmpdir: tmpdir


def kernel(x, shift_param):
    from concourse.bass_utils import run_bass_kernel_spmd

    x = np.asarray(x, dtype=np.float32)
    sp = np.asarray(shift_param, dtype=np.float32)
    assert x.shape == (N, C, H, W)

    perm, taps, runs = _plan(sp)
    wd, wvv = _host_weights(sp, perm, taps)
    xp = np.ascontiguousarray(x[:, perm].astype(np.float16))

    key = (taps, runs)
    if _CACHE.get("key") != key:
        _CACHE["nc"] = _build_nc(taps, runs)
        _CACHE["key"] = key
    nc = _CACHE["nc"]

    in_maps = [{"xs": xp[i * NSH:(i + 1) * NSH], "wd": wd, "wv": wvv}
               for i in range(NCORES)]
    trace = os.environ.get("ASL_TRACE") == "1"
    if trace:
        _install_trace_shim()
    res = run_bass_kernel_spmd(nc, in_maps, list(range(NCORES)), trace=trace)
    if trace:
        print(f"HW exec time: {res.exec_time_ns} ns")
        _CACHE["last_result"] = res
    ysp = np.concatenate([r["ys"] for r in res.results], axis=0)
    out = np.empty((N, C, H, W), np.float32)
    out[:, perm] = ysp.astype(np.float32)
    return out


# revision 7
# speedup vs baseline: 1.3481x; 1.3481x over previous
"""ActiveShiftLayer Trainium2 kernel (v3).

out[n,c,h,w] = bilinear sample of x[n,c, h+alpha_c, w+beta_c], zero outside
the spatial extent.

alpha,beta in [-1,1) => floor in {-1,0}, so the bilinear sample is a
separable 3-tap convolution along H then W with per-channel tap weights;
per channel only 2 of the 3 taps are nonzero in each direction.

Optimizations over the 83.5us v1:
- fp16 input: x is cast to fp16 on the host, halving HBM read traffic
  (error budget 2e-2 >> fp16's ~1e-4).
- channel sort: channels are permuted on the host, grouped by floor(alpha).
  A 128-channel block whose channels share floor(alpha) needs only 2 V-tap
  matmul passes instead of 3. Because per channel only ONE of the two H
  outer taps is nonzero, a single prescaled TMP plane serves both shifted
  adds (w_out = wh_m1 + wh_p1).
- V-stage on TensorE: accumulating fp16 diag matmuls into PSUM (2048-col
  pieces = 4 banks); ScalarE drains PSUM -> VT (fp16 SBUF) - its only
  elementwise pass.
- H-stage entirely on VectorE in fp16 at tile granularity, hitting the
  DVE fast paths: OUT = VT*wh0 and TMP = VT*w_out are tensor_scalar ops
  (4x_2p, 0.25 cyc/elem); the two shifted accumulates are tensor_tensor
  adds (2x_1p, 0.5 cyc/elem) issued per 32-aligned partition range of the
  needed side (other-side channels have exactly-zero TMP, so overlap adds
  0).
- stores are plain fp16 HWDGE per tile triggered from the Scalar queue;
  the sync queue only carries loads.

The Bass program is built after seeing shift_param (tap sets / partition
runs are data-dependent) and cached by that structure.

Sharding: data-parallel over batch (N=32 -> 4 per core), each core also
splits C=256 into two partition blocks -> 8 tiles of [128 channels
(partitions), 56x56 plane (free dim)] per core. Pure SPMD, no collectives.
"""

import os
import numpy as np

N, C, H, W = 32, 256, 56, 56
NCORES = 8
NSH = N // NCORES  # batches per core
P = 128
CB = C // P        # channel blocks
HW = H * W         # 3136
CHUNK = 512        # one PSUM bank of f32 per matmul
XLEN = W + HW + W + 16  # guard row above/below + pad

_CACHE = {}


def _build_nc(taps, runs):
    """taps: per-cb tuple of dy offsets needed (subset of (-1,0,1)).
    runs: per-cb tuple of (p0, p1, side) 32-aligned partition ranges;
    side 0 = left tap (out[w] += tmp[w-1]), side 1 = right tap."""
    import concourse.bacc as bacc
    import concourse.mybir as mybir
    import concourse.tile as tile

    f32 = mybir.dt.float32
    f16 = mybir.dt.float16
    add = mybir.AluOpType.add
    act_copy = mybir.ActivationFunctionType.Copy

    nc = bacc.Bacc()
    xs = nc.dram_tensor("xs", [NSH, C, H, W], f16, kind="ExternalInput")
    NT0 = max(len(t) for t in taps)
    wd = nc.dram_tensor("wd", [CB, P, NT0 * P], f16, kind="ExternalInput")
    # wv[cb] columns: [wh_0, wh_m1, wh_p1]
    wv = nc.dram_tensor("wv", [CB, P, 3], f32, kind="ExternalInput")
    ys = nc.dram_tensor("ys", [NSH, C, H, W], f16, kind="ExternalOutput")

    with tile.TileContext(nc) as tc:
        with tc.tile_pool(name="wp", bufs=1) as wp, \
             tc.tile_pool(name="vt", bufs=3) as vpool, \
             tc.tile_pool(name="tm", bufs=4) as tpool, \
             tc.tile_pool(name="op", bufs=3) as opool, \
             tc.tile_pool(name="ps", bufs=2, space="PSUM") as ppool:

            wdt = []
            wvt = []

            def load_weights(cb):
                t = wp.tile([P, NT0 * P], f16, tag=f"wd{cb}")
                nc.sync.dma_start(t[:], wd[cb])
                wdt.append(t)
                v = wp.tile([P, 3], f32, tag=f"wv{cb}")
                nc.sync.dma_start(v[:], wv[cb])
                wvt.append(v)

            # persistent X buffers: guards zeroed once, loads only rewrite
            # the middle region
            NXBUF = 4
            xbufs = []
            for i in range(NXBUF):
                xb = wp.tile([P, XLEN], f16, tag=f"X{i}")
                nc.vector.memset(xb[:, 0:W], 0.0)
                nc.vector.memset(xb[:, W + HW:W + HW + W], 0.0)
                xbufs.append(xb)

            load_weights(0)

            tiles = [(n, cb) for n in range(NSH) for cb in range(CB)]
            NT = len(tiles)

            # column boundaries of PSUM pieces per tile (<= 2048 cols each)
            def bounds(idx):
                if idx == 0:
                    return [0, 512, 1024, 2048, 3136]
                if idx == NT - 1:
                    return [0, 1024, 2048, 2560, 3136]
                return [0, 2048, 3136]

            def issue_load(idx):
                # segmented so piece i only depends on segments 0..i
                ln, lcb = tiles[idx]
                lcs = slice(lcb * P, (lcb + 1) * P)
                X = xbufs[idx % NXBUF]
                xflat = xs[ln, lcs, :, :].rearrange("p h w -> p (h w)")
                b = bounds(idx)
                cuts = [min(-(-c1 // W) + 1, H) for c1 in b[1:-1]] + [H]
                r0 = 0
                for r1 in cuts:
                    if r1 > r0:
                        nc.sync.dma_start(X[:, W + r0 * W:W + r1 * W],
                                          xflat[:, r0 * W:r1 * W])
                    r0 = r1
                return X

            xtiles = {}
            xtiles[0] = issue_load(0)
            load_weights(1)
            xtiles[1] = issue_load(1)
            xtiles[2] = issue_load(2)

            for tidx, (n, cb) in enumerate(tiles):
                wvc = wvt[cb]
                cs = slice(cb * P, (cb + 1) * P)
                if tidx + 3 < NT:
                    xtiles[tidx + 3] = issue_load(tidx + 3)
                X = xtiles.pop(tidx)

                VT = vpool.tile([P, HW], f16)
                OUT = opool.tile([P, HW], f16)

                tb = bounds(tidx)
                tcb = taps[cb]
                for c0, c1 in zip(tb[:-1], tb[1:]):
                    PZ = c1 - c0
                    # V-stage: accumulating fp16 diag matmuls, taps at row
                    # offsets dy*56 into guarded X
                    PS = ppool.tile([P, 4 * CHUNK], f32, tag="ps")
                    for ti, dy in enumerate(tcb):
                        for k0 in range(0, PZ, CHUNK):
                            cn = min(CHUNK, PZ - k0)
                            o = W + c0 + k0 + dy * W
                            nc.tensor.matmul(
                                PS[:, k0:k0 + cn],
                                wdt[cb][:, ti * P:(ti + 1) * P],
                                X[:, o:o + cn],
                                start=(ti == 0), stop=(ti == len(tcb) - 1))
                    # drain PSUM -> fp16 VT (ScalarE's only pass)
                    nc.scalar.activation(VT[:, c0:c1], PS[:, 0:PZ], act_copy)

                # H-stage on DVE, whole tile, fp16 fast paths.
                # Separate prescaled planes per side: TMP_L/TMP_R are
                # exactly zero on other-side channels, so the 32-aligned
                # range overlap adds 0 there.
                nc.vector.tensor_scalar_mul(OUT[:], VT[:], wvc[:, 0:1])
                o3 = OUT[:].rearrange("p (h w) -> p h w", w=W)
                sides_present = {s for (_, _, s) in runs[cb]}
                t3s = {}
                for s in sorted(sides_present):
                    TMP = tpool.tile([P, HW], f16, tag=f"tmp{s}")
                    nc.vector.tensor_scalar_mul(TMP[:], VT[:],
                                                wvc[:, 1 + s:2 + s])
                    t3s[s] = TMP[:].rearrange("p (h w) -> p h w", w=W)
                for (a, b2, side) in runs[cb]:
                    t3 = t3s[side]
                    if side == 0:
                        nc.vector.tensor_tensor(
                            o3[a:b2, :, 1:W], t3[a:b2, :, 0:W - 1],
                            o3[a:b2, :, 1:W], op=add)
                    else:
                        nc.vector.tensor_tensor(
                            o3[a:b2, :, 0:W - 1], t3[a:b2, :, 1:W],
                            o3[a:b2, :, 0:W - 1], op=add)

                yflat = ys[n, cs, :, :].rearrange("p h w -> p (h w)")
                nc.scalar.dma_start(yflat, OUT[:])
    nc.finalize()
    return nc


def _tap_weights(shift):
    """Per-channel 3-tap weights over offsets {-1,0,1} for shift in [-1,1)."""
    f = np.floor(shift)
    t = (shift - f).astype(np.float32)
    assert np.all((f == -1) | (f == 0)), "shift outside [-1,1) unsupported"
    w_m1 = np.where(f == -1, 1 - t, 0).astype(np.float32)
    w_0 = np.where(f == -1, t, 1 - t).astype(np.float32)
    w_p1 = np.where(f == 0, t, 0).astype(np.float32)
    return w_m1, w_0, w_p1


def _plan(sp):
    """Channel permutation + per-block structure from shift_param."""
    fa = np.floor(sp[:, 0]).astype(np.int64)  # alpha: H shift group
    fb = np.floor(sp[:, 1]).astype(np.int64)  # beta: W shift group
    # sort by (alpha group, beta group); -1 group first
    perm = np.lexsort((fb, fa))
    fa_s, fb_s = fa[perm], fb[perm]

    taps = []
    runs = []
    for cb in range(CB):
        cs = slice(cb * P, (cb + 1) * P)
        g = fa_s[cs]
        t = []
        if np.any(g == -1):
            t += [-1, 0]
        if np.any(g == 0):
            if 0 not in t:
                t.append(0)
            t.append(1)
        taps.append(tuple(t))
        # 32-aligned partition ranges per beta side. A range may include
        # partitions of the other side: their TMP weight is exactly 0,
        # so the shifted add is a no-op there.
        sides = (fb_s[cs] == 0).astype(np.int64)  # 0 = left (fb==-1)
        r = []
        for side in (0, 1):
            blocks = sorted({p // 32 for p in range(P)
                             if sides[p] == side})
            a = None
            prev = None
            for b in blocks + [None]:
                if b is not None and prev is not None and b == prev + 1:
                    prev = b
                    continue
                if a is not None:
                    r.append((a * 32, (prev + 1) * 32, side))
                a = b
                prev = b
        runs.append(tuple(sorted(r)))
    return perm, tuple(taps), tuple(runs)


def _host_weights(sp, perm, taps):
    sps = sp[perm]
    wh_m1, wh_0, wh_p1 = _tap_weights(sps[:, 1])  # beta: W shift
    wv_m1, wv_0, wv_p1 = _tap_weights(sps[:, 0])  # alpha: H shift
    vtap = {-1: wv_m1, 0: wv_0, 1: wv_p1}
    NT0 = max(len(t) for t in taps)
    wd = np.zeros((CB, NT0, P, P), np.float32)
    for cb in range(CB):
        cs = slice(cb * P, (cb + 1) * P)
        for ti, dy in enumerate(taps[cb]):
            wd[cb, ti] = np.diag(vtap[dy][cs])
    wd = wd.transpose(0, 2, 1, 3).reshape(CB, P, NT0 * P)
    wd = np.ascontiguousarray(wd.astype(np.float16))
    wvv = np.stack([wh_0, wh_m1, wh_p1], axis=1).astype(np.float32)
    wvv = np.ascontiguousarray(wvv.reshape(CB, P, 3))
    return wd, wvv


def _install_trace_shim():
    """Dev-only: register the NTFF profile hook this container's antenv lacks,
    and stub out the artifact upload (zero-egress container)."""
    import sys
    import types

    try:
        from antenv.axon_hooks import get_axon_ntff_profile_hook  # noqa: F401
    except ImportError:
        from trn_agent_boot.trn_boot import _ntff_profile_via_ctypes

        hook = _ntff_profile_via_ctypes("/opt/axon/libaxon_pjrt.so")
        mod = types.ModuleType("antenv.axon_hooks")
        mod.get_axon_ntff_profile_hook = lambda: hook
        mod.set_axon_ntff_profile_hook = lambda h: None
        import antenv

        sys.modules["antenv.axon_hooks"] = mod
        antenv.axon_hooks = mod

    import concourse.bass_utils as bu

    bu.upload_artifacts = lambda tmpdir: tmpdir


def kernel(x, shift_param):
    from concourse.bass_utils import run_bass_kernel_spmd

    x = np.asarray(x, dtype=np.float32)
    sp = np.asarray(shift_param, dtype=np.float32)
    assert x.shape == (N, C, H, W)

    perm, taps, runs = _plan(sp)
    wd, wvv = _host_weights(sp, perm, taps)
    xp = np.ascontiguousarray(x[:, perm].astype(np.float16))

    key = (taps, runs)
    if _CACHE.get("key") != key:
        _CACHE["nc"] = _build_nc(taps, runs)
        _CACHE["key"] = key
    nc = _CACHE["nc"]

    in_maps = [{"xs": xp[i * NSH:(i + 1) * NSH], "wd": wd, "wv": wvv}
               for i in range(NCORES)]
    trace = os.environ.get("ASL_TRACE") == "1"
    if trace:
        _install_trace_shim()
    res = run_bass_kernel_spmd(nc, in_maps, list(range(NCORES)), trace=trace)
    if trace:
        print(f"HW exec time: {res.exec_time_ns} ns")
        _CACHE["last_result"] = res
    ysp = np.concatenate([r["ys"] for r in res.results], axis=0)
    out = np.empty((N, C, H, W), np.float32)
    out[:, perm] = ysp.astype(np.float32)
    return out


# revision 9
# speedup vs baseline: 1.5336x; 1.1376x over previous
"""ActiveShiftLayer Trainium2 kernel (v3).

out[n,c,h,w] = bilinear sample of x[n,c, h+alpha_c, w+beta_c], zero outside
the spatial extent.

alpha,beta in [-1,1) => floor in {-1,0}, so the bilinear sample is a
separable 3-tap convolution along H then W with per-channel tap weights;
per channel only 2 of the 3 taps are nonzero in each direction.

Optimizations over the 83.5us v1:
- fp16 input: x is cast to fp16 on the host, halving HBM read traffic
  (error budget 2e-2 >> fp16's ~1e-4).
- channel sort: channels are permuted on the host, grouped by floor(alpha).
  A 128-channel block whose channels share floor(alpha) needs only 2 V-tap
  matmul passes instead of 3. Because per channel only ONE of the two H
  outer taps is nonzero, a single prescaled TMP plane serves both shifted
  adds (w_out = wh_m1 + wh_p1).
- V-stage on TensorE: accumulating fp16 diag matmuls into PSUM (2048-col
  pieces = 4 banks); ScalarE drains PSUM -> VT (fp16 SBUF) - its only
  elementwise pass.
- H-stage entirely on VectorE in fp16 at tile granularity, hitting the
  DVE fast paths: OUT = VT*wh0 and TMP = VT*w_out are tensor_scalar ops
  (4x_2p, 0.25 cyc/elem); the two shifted accumulates are tensor_tensor
  adds (2x_1p, 0.5 cyc/elem) issued per 32-aligned partition range of the
  needed side (other-side channels have exactly-zero TMP, so overlap adds
  0).
- stores are plain fp16 HWDGE per tile triggered from the Scalar queue;
  the sync queue only carries loads.

The Bass program is built after seeing shift_param (tap sets / partition
runs are data-dependent) and cached by that structure.

Sharding: data-parallel over batch (N=32 -> 4 per core), each core also
splits C=256 into two partition blocks -> 8 tiles of [128 channels
(partitions), 56x56 plane (free dim)] per core. Pure SPMD, no collectives.
"""

import os
import numpy as np

N, C, H, W = 32, 256, 56, 56
NCORES = 8
NSH = N // NCORES  # batches per core
P = 128
CB = C // P        # channel blocks
HW = H * W         # 3136
CHUNK = 512        # one PSUM bank of f32 per matmul
XLEN = W + HW + W + 16  # guard row above/below + pad

_CACHE = {}


def _build_nc(taps, runs, peh):
    """taps: per-cb tuple of dy offsets needed (subset of (-1,0,1)).
    runs: per-cb tuple of (p0, p1, side) partition ranges; side 0 = left
    tap (out[w] += tmp[w-1]), side 1 = right tap. peh: tile indices whose
    H-stage runs on TensorE (flat taps over guarded VT + wrap fixups)."""
    import concourse.bacc as bacc
    import concourse.mybir as mybir
    import concourse.tile as tile

    f32 = mybir.dt.float32
    f16 = mybir.dt.float16
    add = mybir.AluOpType.add
    mult = mybir.AluOpType.mult
    act_copy = mybir.ActivationFunctionType.Copy

    nc = bacc.Bacc()
    xs = nc.dram_tensor("xs", [NSH, C, H, W], f16, kind="ExternalInput")
    NT0 = max(len(t) for t in taps)
    # V-tap diags followed by 3 H-tap diags (wh_m1, wh_0, wh_p1)
    wd = nc.dram_tensor("wd", [CB, P, (NT0 + 3) * P], f16,
                        kind="ExternalInput")
    # wv[cb] columns: [wh_0, wh_m1, wh_p1, -wh_m1, -wh_p1]
    wv = nc.dram_tensor("wv", [CB, P, 5], f32, kind="ExternalInput")
    ys = nc.dram_tensor("ys", [NSH, C, H, W], f16, kind="ExternalOutput")

    with tile.TileContext(nc) as tc:
        with tc.tile_pool(name="wp", bufs=1) as wp, \
             tc.tile_pool(name="vt", bufs=3) as vpool, \
             tc.tile_pool(name="tm", bufs=4) as tpool, \
             tc.tile_pool(name="op", bufs=3) as opool, \
             tc.tile_pool(name="ps", bufs=2, space="PSUM") as ppool:

            wdt = []
            wvt = []

            def load_weights(cb):
                t = wp.tile([P, (NT0 + 3) * P], f16, tag=f"wd{cb}")
                nc.sync.dma_start(t[:], wd[cb])
                wdt.append(t)
                v = wp.tile([P, 5], f32, tag=f"wv{cb}")
                nc.sync.dma_start(v[:], wv[cb])
                wvt.append(v)

            # persistent X buffers: guards zeroed once, loads only rewrite
            # the middle region
            NXBUF = 4
            xbufs = []
            for i in range(NXBUF):
                xb = wp.tile([P, XLEN], f16, tag=f"X{i}")
                nc.vector.memset(xb[:, 0:W], 0.0)
                nc.vector.memset(xb[:, W + HW:W + HW + W], 0.0)
                xbufs.append(xb)
            # persistent VT buffers with 1-element guards at 0 and 1+HW
            # for the PE H-stage flat taps
            NVBUF = 3
            vbufs = []
            for i in range(NVBUF):
                vb = wp.tile([P, 3200], f16, tag=f"V{i}")
                nc.vector.memset(vb[:, 0:1], 0.0)
                nc.vector.memset(vb[:, 1 + HW:2 + HW], 0.0)
                vbufs.append(vb)

            load_weights(0)

            tiles = [(n, cb) for n in range(NSH) for cb in range(CB)]
            NT = len(tiles)

            # column boundaries of PSUM pieces per tile (<= 2048 cols each)
            def bounds(idx):
                if idx == 0:
                    return [0, 512, 1024, 2048, 3136]
                if idx == NT - 1:
                    return [0, 1024, 2048, 2560, 3136]
                return [0, 2048, 3136]

            def issue_load(idx):
                # segmented so piece i only depends on segments 0..i
                ln, lcb = tiles[idx]
                lcs = slice(lcb * P, (lcb + 1) * P)
                X = xbufs[idx % NXBUF]
                xflat = xs[ln, lcs, :, :].rearrange("p h w -> p (h w)")
                b = bounds(idx)
                cuts = [min(-(-c1 // W) + 1, H) for c1 in b[1:-1]] + [H]
                r0 = 0
                for r1 in cuts:
                    if r1 > r0:
                        nc.sync.dma_start(X[:, W + r0 * W:W + r1 * W],
                                          xflat[:, r0 * W:r1 * W])
                    r0 = r1
                return X

            xtiles = {}
            xtiles[0] = issue_load(0)
            load_weights(1)
            xtiles[1] = issue_load(1)
            xtiles[2] = issue_load(2)

            for tidx, (n, cb) in enumerate(tiles):
                wvc = wvt[cb]
                cs = slice(cb * P, (cb + 1) * P)
                if tidx + 3 < NT:
                    xtiles[tidx + 3] = issue_load(tidx + 3)
                X = xtiles.pop(tidx)

                Vb = vbufs[tidx % NVBUF]
                VT = Vb[:, 1:1 + HW]
                OUT = opool.tile([P, HW], f16)

                tb = bounds(tidx)
                tcb = taps[cb]
                for c0, c1 in zip(tb[:-1], tb[1:]):
                    PZ = c1 - c0
                    # V-stage: accumulating fp16 diag matmuls, taps at row
                    # offsets dy*56 into guarded X
                    PS = ppool.tile([P, 4 * CHUNK], f32, tag="ps")
                    for ti, dy in enumerate(tcb):
                        for k0 in range(0, PZ, CHUNK):
                            cn = min(CHUNK, PZ - k0)
                            o = W + c0 + k0 + dy * W
                            nc.tensor.matmul(
                                PS[:, k0:k0 + cn],
                                wdt[cb][:, ti * P:(ti + 1) * P],
                                X[:, o:o + cn],
                                start=(ti == 0), stop=(ti == len(tcb) - 1))
                    # drain PSUM -> fp16 VT (ScalarE's only pass)
                    nc.scalar.activation(Vb[:, 1 + c0:1 + c1], PS[:, 0:PZ],
                                         act_copy)

                o3 = OUT[:].rearrange("p (h w) -> p h w", w=W)
                if tidx in peh:
                    # H-stage on TensorE: 3 flat taps over guarded VT;
                    # wrapped columns corrected by two tiny STTs after
                    for c0, c1 in zip(tb[:-1], tb[1:]):
                        PZ = c1 - c0
                        PS2 = ppool.tile([P, 4 * CHUNK], f32, tag="ps")
                        for ti in range(3):
                            for k0 in range(0, PZ, CHUNK):
                                cn = min(CHUNK, PZ - k0)
                                o = c0 + k0 + ti  # dx = ti - 1, +1 guard
                                nc.tensor.matmul(
                                    PS2[:, k0:k0 + cn],
                                    wdt[cb][:, (NT0 + ti) * P:
                                             (NT0 + ti + 1) * P],
                                    Vb[:, o:o + cn],
                                    start=(ti == 0), stop=(ti == 2))
                        nc.scalar.activation(OUT[:, c0:c1], PS2[:, 0:PZ],
                                             act_copy)
                    vg0 = Vb[:, 0:HW].rearrange(
                        "p (h w) -> p h w", w=W)[:, :, 0]
                    vg55 = Vb[:, 57:57 + HW].rearrange(
                        "p (h w) -> p h w", w=W)[:, :, 0]
                    nc.vector.scalar_tensor_tensor(
                        o3[:, :, 0], vg0, wvc[:, 3:4], o3[:, :, 0],
                        op0=mult, op1=add)
                    nc.vector.scalar_tensor_tensor(
                        o3[:, :, W - 1], vg55, wvc[:, 4:5],
                        o3[:, :, W - 1], op0=mult, op1=add)
                else:
                    # H-stage on DVE, whole tile, fp16 fast paths.
                    # Separate prescaled planes per side: TMP_L/TMP_R are
                    # exactly zero on other-side channels, so each
                    # full-range shifted add is a no-op there.
                    nc.vector.tensor_scalar_mul(OUT[:], VT, wvc[:, 0:1])
                    t3s = {}
                    for (_, _, s) in runs[cb]:
                        TMP = tpool.tile([P, HW], f16, tag=f"tmp{s}")
                        nc.vector.tensor_scalar_mul(TMP[:], VT,
                                                    wvc[:, 1 + s:2 + s])
                        t3s[s] = TMP[:].rearrange("p (h w) -> p h w", w=W)
                    for (a, b2, side) in runs[cb]:
                        t3 = t3s[side]
                        if side == 0:
                            nc.vector.tensor_tensor(
                                o3[a:b2, :, 1:W], t3[a:b2, :, 0:W - 1],
                                o3[a:b2, :, 1:W], op=add)
                        else:
                            nc.vector.tensor_tensor(
                                o3[a:b2, :, 0:W - 1], t3[a:b2, :, 1:W],
                                o3[a:b2, :, 0:W - 1], op=add)

                yflat = ys[n, cs, :, :].rearrange("p h w -> p (h w)")
                nc.scalar.dma_start(yflat, OUT[:])
    nc.finalize()
    return nc


def _tap_weights(shift):
    """Per-channel 3-tap weights over offsets {-1,0,1} for shift in [-1,1)."""
    f = np.floor(shift)
    t = (shift - f).astype(np.float32)
    assert np.all((f == -1) | (f == 0)), "shift outside [-1,1) unsupported"
    w_m1 = np.where(f == -1, 1 - t, 0).astype(np.float32)
    w_0 = np.where(f == -1, t, 1 - t).astype(np.float32)
    w_p1 = np.where(f == 0, t, 0).astype(np.float32)
    return w_m1, w_0, w_p1


def _plan(sp):
    """Channel permutation + per-block structure from shift_param."""
    fa = np.floor(sp[:, 0]).astype(np.int64)  # alpha: H shift group
    fb = np.floor(sp[:, 1]).astype(np.int64)  # beta: W shift group
    # sort by (alpha group, beta group); -1 group first
    perm = np.lexsort((fb, fa))
    fa_s, fb_s = fa[perm], fb[perm]

    taps = []
    runs = []
    for cb in range(CB):
        cs = slice(cb * P, (cb + 1) * P)
        g = fa_s[cs]
        t = []
        if np.any(g == -1):
            t += [-1, 0]
        if np.any(g == 0):
            if 0 not in t:
                t.append(0)
            t.append(1)
        taps.append(tuple(t))
        # one full-partition-range shifted add per side present; TMP_L/R
        # are exactly zero on other-side channels, so each add is a no-op
        # there.
        sides = (fb_s[cs] == 0).astype(np.int64)  # 0 = left (fb==-1)
        r = [(0, P, s) for s in (0, 1) if np.any(sides == s)]
        runs.append(tuple(r))
    return perm, tuple(taps), tuple(runs)


def _host_weights(sp, perm, taps):
    sps = sp[perm]
    wh_m1, wh_0, wh_p1 = _tap_weights(sps[:, 1])  # beta: W shift
    wv_m1, wv_0, wv_p1 = _tap_weights(sps[:, 0])  # alpha: H shift
    vtap = {-1: wv_m1, 0: wv_0, 1: wv_p1}
    NT0 = max(len(t) for t in taps)
    wd = np.zeros((CB, NT0 + 3, P, P), np.float32)
    for cb in range(CB):
        cs = slice(cb * P, (cb + 1) * P)
        for ti, dy in enumerate(taps[cb]):
            wd[cb, ti] = np.diag(vtap[dy][cs])
        for ti, wh in enumerate((wh_m1, wh_0, wh_p1)):
            wd[cb, NT0 + ti] = np.diag(wh[cs])
    wd = wd.transpose(0, 2, 1, 3).reshape(CB, P, (NT0 + 3) * P)
    wd = np.ascontiguousarray(wd.astype(np.float16))
    wvv = np.stack([wh_0, wh_m1, wh_p1, -wh_m1, -wh_p1],
                   axis=1).astype(np.float32)
    wvv = np.ascontiguousarray(wvv.reshape(CB, P, 5))
    return wd, wvv


def _install_trace_shim():
    """Dev-only: register the NTFF profile hook this container's antenv lacks,
    and stub out the artifact upload (zero-egress container)."""
    import sys
    import types

    try:
        from antenv.axon_hooks import get_axon_ntff_profile_hook  # noqa: F401
    except ImportError:
        from trn_agent_boot.trn_boot import _ntff_profile_via_ctypes

        hook = _ntff_profile_via_ctypes("/opt/axon/libaxon_pjrt.so")
        mod = types.ModuleType("antenv.axon_hooks")
        mod.get_axon_ntff_profile_hook = lambda: hook
        mod.set_axon_ntff_profile_hook = lambda h: None
        import antenv

        sys.modules["antenv.axon_hooks"] = mod
        antenv.axon_hooks = mod

    import concourse.bass_utils as bu

    bu.upload_artifacts = lambda tmpdir: tmpdir


def kernel(x, shift_param):
    from concourse.bass_utils import run_bass_kernel_spmd

    x = np.asarray(x, dtype=np.float32)
    sp = np.asarray(shift_param, dtype=np.float32)
    assert x.shape == (N, C, H, W)

    perm, taps, runs = _plan(sp)
    wd, wvv = _host_weights(sp, perm, taps)
    xp = np.ascontiguousarray(x[:, perm].astype(np.float16))

    npeh = int(os.environ.get("ASL_PEH", "2"))
    nt = NSH * CB
    peh = frozenset(range(nt - npeh, nt)) if npeh else frozenset()
    key = (taps, runs, peh)
    if _CACHE.get("key") != key:
        _CACHE["nc"] = _build_nc(taps, runs, peh)
        _CACHE["key"] = key
    nc = _CACHE["nc"]

    in_maps = [{"xs": xp[i * NSH:(i + 1) * NSH], "wd": wd, "wv": wvv}
               for i in range(NCORES)]
    trace = os.environ.get("ASL_TRACE") == "1"
    if trace:
        _install_trace_shim()
    res = run_bass_kernel_spmd(nc, in_maps, list(range(NCORES)), trace=trace)
    if trace:
        print(f"HW exec time: {res.exec_time_ns} ns")
        _CACHE["last_result"] = res
    ysp = np.concatenate([r["ys"] for r in res.results], axis=0)
    out = np.empty((N, C, H, W), np.float32)
    out[:, perm] = ysp.astype(np.float32)
    return out
